# revision 46
# baseline (speedup 1.0000x reference)
"""BERT multi-head self-attention on 8 Trainium2 NeuronCores.

Problem: B=2, S=2048, H=768, NH=12, HD=64 (fp32 reference).

Sharding (hardcoded): core c in 0..7 handles batch b=c//4 and head group
g=c%4 (heads 3g..3g+2).  Each core computes its 3 heads' attention plus the
partial output projection ctx_g @ Wo[rows of g]; the host sums the 4 partial
outputs per batch element and adds the (bv @ Wo + bo) constant row.

Device pipeline per core (all matmuls on PE, exp on ACT, evictions on DVE):
  1. QKT = Wqk^T-ish projection producing Q^T/K^T in [head_dim, seq] layout
     (so the scores matmul needs no transposes), with bias folded into the
     PSUM->SBUF eviction.
  2. V natural [seq, head_dim] with a ones-augmented column per head (the
     column carries the attention-mask multiplier), so the P@V_aug matmul
     yields both ctx^T and the softmax denominator.
  3. scoresT[k, q] = K^T.T @ Q^T per 128-row k-block; ACT computes
     exp(0.125*scores) straight out of PSUM.
  4. ctxT_aug[d+1, q] accumulates over k-blocks in PSUM; row 64 is the
     denominator.  Reciprocal, then a partition-broadcast of 1/denom (bounced
     through a DRAM scratch row, since stride-0-partition DMA reads are only
     legal from DRAM) and a DVE multiply normalize ctx^T.
  5. out[q, hout] = ctxT_norm.T @ Wo_slice per 128-row q-block.

Scheduling: the ScalarE exp stream (~100us/core) is the binding resource, so
the program is one flat software pipeline over all (head, q-half, k-block)
units in which scores/exp lead their P@V consumer (by 1 slot in the first two
blocks, by 3 afterwards so the PV that waits on the previous block's
normalize chain never starves ACT).  Only a 5-unit projection prefix runs
before the first exp; the remaining QKV-projection work is drained one unit
per iteration inside the first two blocks, in deadline order, borrowing
scores-PSUM slots.  PSUM is packed exactly to its 8 banks (3 score buffers x
2 banks + 1 ctx accumulator x 2 banks).
"""

import os
import sys
import numpy as np

for _p in ("/opt/trn_rl_repo",):
    if _p not in sys.path and os.path.isdir(_p):
        sys.path.append(_p)

import ml_dtypes  # noqa: E402

from concourse import bacc  # noqa: E402
import concourse.mybir as mybir  # noqa: E402
import concourse.tile as tile  # noqa: E402
from concourse.bass_utils import run_bass_kernel_spmd  # noqa: E402

B, S, H = 2, 2048, 768
NH, HD = 12, 64
HPC = 3            # heads per core
NCORES = 8
P = 128
NKB = S // P       # 16 k-blocks
NQB = S // P       # 16 q-blocks
NHC = H // P       # 6 contraction chunks over hidden dim
QH = 1024          # q-half size for the scores/exp/PV pipeline
NQH = S // QH      # 2
F32 = mybir.dt.float32

CDT = mybir.dt.bfloat16   # compute dtype for matmul operands
NP_CDT = ml_dtypes.bfloat16


def _build_nc(use_mask: bool):
    import contextlib

    nc = bacc.Bacc("TRN2", target_bir_lowering=False)
    AF = mybir.ActivationFunctionType

    xt = nc.dram_tensor("xt", [H, S], CDT, kind="ExternalInput")
    # wqk columns ordered [Q0|Q1|K0|K1|Q2|K2] (64 cols each)
    wqk = nc.dram_tensor("wqk", [H, 2 * HPC * HD], CDT, kind="ExternalInput")
    wv = nc.dram_tensor("wv", [H, HPC * HD], CDT, kind="ExternalInput")
    wo = nc.dram_tensor("wo", [HPC * HD, H], CDT, kind="ExternalInput")
    # bqk rows ordered to match wqk columns
    bqk = nc.dram_tensor("bqk", [2 * HPC * HD, 1], F32, kind="ExternalInput")
    if use_mask:
        mv = nc.dram_tensor("mv", [S, 1], F32, kind="ExternalInput")
    out = nc.dram_tensor("out", [S, H], CDT, kind="ExternalOutput")
    rspill = nc.dram_tensor("rspill", [HPC * NQH, QH], F32)  # internal scratch

    with tile.TileContext(nc) as tc, contextlib.ExitStack() as ctx, \
            nc.allow_low_precision(reason="bf16 compute pipeline by design"):
        const = ctx.enter_context(tc.tile_pool(name="const", bufs=1))
        xt_pool = ctx.enter_context(tc.tile_pool(name="xt", bufs=1))
        w_pool = ctx.enter_context(tc.tile_pool(name="w", bufs=1))
        qkt_pool = ctx.enter_context(tc.tile_pool(name="qkt", bufs=1))
        v_pool = ctx.enter_context(tc.tile_pool(name="v", bufs=1))
        pt_pool = ctx.enter_context(tc.tile_pool(name="pt", bufs=6))
        ctxu_pool = ctx.enter_context(tc.tile_pool(name="ctxu", bufs=1))
        ctxn_pool = ctx.enter_context(tc.tile_pool(name="ctxn", bufs=1))
        out_sb_pool = ctx.enter_context(tc.tile_pool(name="outsb", bufs=6))

        # ---- load inputs (weights first, single consolidated DMAs; xt
        # column-major across both HWDGE rings so compute starts as soon as
        # the first 512 q-columns of every h-chunk land) ----
        wqk_t = w_pool.tile([P, NHC, 2 * HPC * HD], CDT, tag="wqk")
        nc.scalar.dma_start(
            wqk_t[:], wqk[:].rearrange("(c p) n -> p c n", p=P))
        wqk_sb = [wqk_t[:, c, :] for c in range(NHC)]
        wv_t = w_pool.tile([P, NHC, HPC * HD], CDT, tag="wv")
        nc.scalar.dma_start(
            wv_t[:], wv[:].rearrange("(c p) n -> p c n", p=P))
        wv_sb = [wv_t[:, c, :] for c in range(NHC)]
        bias_t = const.tile([P, 3], F32, tag="bqk")
        nc.scalar.dma_start(
            bias_t[:], bqk[:].rearrange("(m p) one -> p (m one)", p=P))
        bias_sb = [bias_t[:, m:m + 1] for m in range(3)]
        if use_mask:
            mv_t = const.tile([P, NKB], F32, tag="mv")
            nc.scalar.dma_start(
                mv_t[:], mv[:].rearrange("(kb p) one -> p (kb one)", p=P))
            mv_sb = [mv_t[:, kb:kb + 1] for kb in range(NKB)]
        xt_sb = [xt_pool.tile([P, S], CDT, tag=f"xt{c}", name=f"xtsb{c}")
                 for c in range(NHC)]
        for qg in range(2):  # [128, 1024] pieces; first q-half on sync ring
            for c in range(NHC):
                qs = slice(qg * QH, (qg + 1) * QH)
                eng = nc.sync if qg % 2 == 0 else nc.scalar
                eng.dma_start(xt_sb[c][:, qs], xt[c * P:(c + 1) * P, qs])
        wo_t = w_pool.tile([HD, HPC, H], CDT, tag="wo")
        nc.scalar.dma_start(
            wo_t[:], wo[:].rearrange("(h p) n -> p h n", p=HD))
        wo_sb = [wo_t[:, h, :] for h in range(HPC)]

        # ---- QKT projection (m-blocks [Q0|Q1], [K0|K1], [Q2|K2]) and V,
        # interleaved qc-major so each 512-column group only waits for its
        # own slice of the xt DMA ----
        tq01 = qkt_pool.tile([P, S], CDT, tag="tq01")
        tk01 = qkt_pool.tile([P, S], CDT, tag="tk01")
        tqk2 = qkt_pool.tile([P, S], CDT, tag="tqk2")
        qkt_tiles = [tq01, tk01, tqk2]
        v_sb = [None] * NKB

        def qkt_unit(psum_tile_fn, qc, m):
            qs = slice(qc * 512, (qc + 1) * 512)
            ps = psum_tile_fn([P, 512], "qkps")
            for c in range(NHC):
                nc.tensor.matmul(
                    ps[:],
                    wqk_sb[c][:, m * P:(m + 1) * P],
                    xt_sb[c][:, qs],
                    start=(c == 0), stop=(c == NHC - 1),
                )
            nc.vector.tensor_scalar_add(
                qkt_tiles[m][:, qs], ps[:], bias_sb[m][:]
            )

        def v_unit(psum_tile_fn, kb):
            ps = psum_tile_fn([P, HPC * HD], "vps")
            for c in range(NHC):
                nc.tensor.matmul(
                    ps[:],
                    xt_sb[c][:, kb * P:(kb + 1) * P],
                    wv_sb[c][:],
                    start=(c == 0), stop=(c == NHC - 1),
                )
            vt = v_pool.tile([P, HPC, HD + 1], CDT, tag=f"v{kb}",
                             name=f"vt{kb}")
            nc.vector.tensor_copy(
                vt[:, :, 0:HD], ps[:].rearrange("p (h d) -> p h d", h=HPC)
            )
            nc.vector.memset(vt[:, :, HD:HD + 1], 1.0)
            if use_mask:
                nc.vector.tensor_scalar_mul(vt[:], vt[:], mv_sb[kb][:])
            v_sb[kb] = vt

        # Minimal prefix: just enough for attention block (qh0, h0) to
        # start — Q0/K0 over the first q-half plus V(0).  Everything else is
        # emitted as fillers inside blocks 0-1, one unit per iteration, in
        # deadline order (V(kb) one iteration ahead of its PV; K-columns one
        # k-group ahead of their scores).
        with tc.tile_pool(name="qkt_psum", bufs=2, space="PSUM") as qkt_psum:
            def pre_tile(shape, name):
                return qkt_psum.tile(shape, F32, tag="qkt", name=name)
            qkt_unit(pre_tile, 0, 0)
            qkt_unit(pre_tile, 0, 1)
            qkt_unit(pre_tile, 1, 0)
            qkt_unit(pre_tile, 1, 1)
            v_unit(pre_tile, 0)
        # K2 lives at rows 64-127 of tqk2; it must move to rows 0-63 of its
        # own tile (engines cannot shift partitions; DMA can).  The copy is
        # emitted at the end of attention block (qh0, h0), once tqk2 is done.
        tk2 = qkt_pool.tile([HD, S], CDT, tag="tk2")

        def q_ap(h, sl):  # Q_h^T [64, sl] at base partition 0 or 64
            if h == 0:
                return tq01[0:HD, sl]
            if h == 1:
                return tq01[HD:2 * HD, sl]
            return tqk2[0:HD, sl]

        def k_ap(h, sl):  # K_h^T [64, sl], base partition matching q_ap
            if h == 0:
                return tk01[0:HD, sl]
            if h == 1:
                return tk01[HD:2 * HD, sl]
            return tk2[0:HD, sl]

        # ---- attention per head ----
        # ctxu / ctxn tiles: per head [64(+1), S] at base partition 0
        ctxu_t = [ctxu_pool.tile([HD, S], F32, tag=f"ctxu{h}", name=f"ctxu{h}")
                  for h in range(HPC)]
        ctxn_t = [ctxn_pool.tile([HD, S], CDT, tag=f"ctxn{h}", name=f"ctxn{h}")
                  for h in range(HPC)]
        recip_t = [ctxu_pool.tile([65, S], F32, tag=f"recip{h}", name=f"recip{h}")
                   for h in range(HPC)]
        rbc_pool = ctx.enter_context(tc.tile_pool(name="rbc", bufs=2))

        def op_unit(psum_tile_fn, qb):
            qsl = slice(qb * P, (qb + 1) * P)
            ops = psum_tile_fn([P, H], "ops")
            for nchunk in range(2):
                nsl = slice(nchunk * 512, min((nchunk + 1) * 512, H))
                for h in range(HPC):
                    nc.tensor.matmul(
                        ops[:, nsl],
                        ctxn_t[h][:, qsl],
                        wo_sb[h][:, nsl],
                        start=(h == 0), stop=(h == HPC - 1),
                    )
            osb = out_sb_pool.tile([P, H], CDT, tag="osb", name="osb")
            if qb % 2 == 0:
                nc.vector.tensor_copy(osb[:], ops[:])
                nc.sync.dma_start(out[qsl, :], osb[:])
            else:
                nc.scalar.copy(osb[:], ops[:])
                nc.scalar.dma_start(out[qsl, :], osb[:])

        with tc.tile_pool(name="sc_psum", bufs=3, space="PSUM") as sc_psum, \
             tc.tile_pool(name="ctx_psum", bufs=1, space="PSUM") as ctx_psum:
            def sc_tile(shape, name):
                return sc_psum.tile(shape, F32, tag="sc", name=name)

            # remaining projection work in deadline order, borrowing scores
            # psum slots, consumed one unit per iteration in blocks 0-1
            sched = ([("v", kb) for kb in (1, 2, 3, 4, 5, 6)] + [("qk", 2, 1)]
                     + [("v", kb) for kb in (7, 8, 9, 10)] + [("qk", 3, 1)]
                     + [("v", kb) for kb in (11, 12, 13, 14, 15)]
                     + [("qk", 2, 2), ("qk", 3, 2), ("qk", 0, 2),
                        ("qk", 1, 2), ("qk", 2, 0), ("qk", 3, 0)])
            fillers = []
            for u in sched:
                if u[0] == "v":
                    fillers.append(lambda kb=u[1]: v_unit(sc_tile, kb))
                else:
                    fillers.append(
                        lambda qc=u[1], m=u[2]: qkt_unit(sc_tile, qc, m))
            fillers.reverse()  # consume with .pop()

            blocks = [(qh, h) for qh in range(NQH) for h in range(HPC)]

            def scores(bi, kb):
                qh, h = blocks[bi]
                ksl = slice(kb * P, (kb + 1) * P)
                sps = sc_psum.tile([P, QH], F32, tag="sc", name="sps")
                for c in range(QH // 512):
                    nc.tensor.matmul(
                        sps[:, c * 512:(c + 1) * 512],
                        k_ap(h, ksl),
                        q_ap(h, slice(qh * QH + c * 512,
                                      qh * QH + (c + 1) * 512)),
                        start=True, stop=True,
                    )
                pt = pt_pool.tile([P, QH], CDT, tag="pt", name="pt")
                nc.scalar.activation(pt[:], sps[:], AF.Exp, scale=0.125)
                return pt

            def pv(bi, kb, pt, cps):
                _, h = blocks[bi]
                for c in range(QH // 512):
                    nc.tensor.matmul(
                        cps[:, c * 512:(c + 1) * 512],
                        v_sb[kb][:, h, :],
                        pt[:, c * 512:(c + 1) * 512],
                        start=(kb == 0), stop=(kb == NKB - 1),
                    )

            def normalize(bi, cps):
                qh, h = blocks[bi]
                qsl = slice(qh * QH, (qh + 1) * QH)
                nc.vector.tensor_copy(ctxu_t[h][:, qsl], cps[0:HD, :])
                nc.vector.reciprocal(
                    recip_t[h][HD:HD + 1, qsl], cps[HD:HD + 1, :]
                )
                # broadcast 1/denom across the 64 head-dim partitions:
                # bounce through DRAM (stride-0-partition reads are only
                # legal from DRAM), then multiply
                row = qh * HPC + h
                nc.sync.dma_start(rspill[row, :], recip_t[h][HD:HD + 1, qsl])
                rbc = rbc_pool.tile([HD, QH], F32, tag="rbc", name="rbc")
                nc.sync.dma_start(
                    rbc[:], rspill[row:row + 1, :].to_broadcast((HD, QH)))
                nc.vector.tensor_mul(ctxn_t[h][:, qsl], ctxu_t[h][:, qsl],
                                     rbc[:])

            # one flat software pipeline over all (block, k-block) pairs —
            # scores/exp lead their PV (by 1 slot in blocks 0-1 where fillers
            # provide the PE cushion, by 3 afterwards) even across block
            # boundaries, so ACT never waits at a boundary or on the
            # normalize chain of the previous block
            from collections import deque
            cps_of = {}
            pending = deque()
            gi = 0

            def drain_one():
                pbi, pkb, ppt = pending.popleft()
                pv(pbi, pkb, ppt, cps_of[pbi])
                if pkb == NKB - 1:
                    normalize(pbi, cps_of[pbi])

            for bi in range(len(blocks)):
                for kb in range(NKB):
                    if kb == 0:
                        cps_of[bi] = ctx_psum.tile(
                            [HD + 1, QH], F32, tag="ctx", name=f"cps{bi}")
                        if bi == 2:
                            # K2 partition shift; tqk2 is complete by now
                            nc.sync.dma_start(tk2[:], tqk2[HD:2 * HD, :])
                    pending.append((bi, kb, scores(bi, kb)))
                    gi += 1
                    for _ in range(2 if gi <= 2 else 1):
                        if fillers:
                            fillers.pop()()
                    lag = 1 if fillers else 3
                    while len(pending) > lag:
                        drain_one()
            while pending:
                drain_one()

        # ---- output projection tail ----
        with tc.tile_pool(name="op_psum", bufs=4, space="PSUM") as op_psum:
            def op_tile(shape, name):
                return op_psum.tile(shape, F32, tag="op", name=name)
            for qb in range(NQB):
                op_unit(op_tile, qb)

    nc.compile()
    return nc


_NC_CACHE = {}


def _get_nc(use_mask: bool):
    if use_mask not in _NC_CACHE:
        _NC_CACHE[use_mask] = _build_nc(use_mask)
    return _NC_CACHE[use_mask]


def _shard_inputs(hidden_states, attention_mask, Wq, bq, Wk, bk, Wv, bv, Wo, bo,
                  use_mask):
    """Build the 8 per-core input maps (all host-side numpy)."""
    in_maps = []
    for c in range(NCORES):
        b, g = divmod(c, NCORES // B)
        cols = slice(g * HPC * HD, (g + 1) * HPC * HD)
        # wqk columns ordered [Q0|Q1|K0|K1|Q2|K2] within the group
        wq_g = Wq[:, cols]
        wk_g = Wk[:, cols]
        qk_cols = [wq_g[:, 0:HD], wq_g[:, HD:2 * HD],
                   wk_g[:, 0:HD], wk_g[:, HD:2 * HD],
                   wq_g[:, 2 * HD:3 * HD], wk_g[:, 2 * HD:3 * HD]]
        wqk = np.concatenate(qk_cols, axis=1)
        bq_g = bq[cols]
        bk_g = bk[cols]
        bqk = np.concatenate([bq_g[0:HD], bq_g[HD:2 * HD],
                              bk_g[0:HD], bk_g[HD:2 * HD],
                              bq_g[2 * HD:3 * HD], bk_g[2 * HD:3 * HD]])
        m = {
            "xt": np.ascontiguousarray(hidden_states[b].T).astype(NP_CDT),
            "wqk": np.ascontiguousarray(wqk).astype(NP_CDT),
            "wv": np.ascontiguousarray(Wv[:, cols]).astype(NP_CDT),
            "wo": np.ascontiguousarray(Wo[cols, :]).astype(NP_CDT),
            "bqk": bqk.astype(np.float32).reshape(-1, 1),
        }
        if use_mask:
            mvec = np.exp(-10000.0 * (1.0 - attention_mask[b].astype(np.float64)))
            m["mv"] = mvec.astype(np.float32).reshape(-1, 1)
        in_maps.append(m)
    return in_maps


def kernel(hidden_states, attention_mask, Wq, bq, Wk, bk, Wv, bv, Wo, bo):
    hidden_states = np.asarray(hidden_states, np.float32)
    attention_mask = np.asarray(attention_mask)
    Wq, bq = np.asarray(Wq, np.float32), np.asarray(bq, np.float32)
    Wk, bk = np.asarray(Wk, np.float32), np.asarray(bk, np.float32)
    Wv, bv = np.asarray(Wv, np.float32), np.asarray(bv, np.float32)
    Wo, bo = np.asarray(Wo, np.float32), np.asarray(bo, np.float32)

    use_mask = not bool(np.all(attention_mask == 1))
    nc = _get_nc(use_mask)
    in_maps = _shard_inputs(hidden_states, attention_mask,
                            Wq, bq, Wk, bk, Wv, bv, Wo, bo, use_mask)
    res = run_bass_kernel_spmd(nc, in_maps, core_ids=list(range(NCORES)))

    # unshard: sum the 4 head-group partials per batch; add constant row.
    const_row = (bv.astype(np.float64) @ Wo.astype(np.float64)
                 + bo.astype(np.float64))
    out = np.zeros((B, S, H), np.float64)
    for c in range(NCORES):
        b = c // (NCORES // B)
        out[b] += res.results[c]["out"].astype(np.float64)
    out += const_row[None, None, :]
    return out.astype(np.float32)


if __name__ == "__main__":
    # smoke test with random data
    rng = np.random.default_rng(0)
    inputs = {
        "hidden_states": rng.standard_normal((B, S, H), np.float32),
        "attention_mask": np.ones((B, S), np.int32),
        "Wq": rng.standard_normal((H, H), np.float32) * 0.02,
        "bq": np.zeros(H, np.float32),
        "Wk": rng.standard_normal((H, H), np.float32) * 0.02,
        "bk": np.zeros(H, np.float32),
        "Wv": rng.standard_normal((H, H), np.float32) * 0.02,
        "bv": np.zeros(H, np.float32),
        "Wo": rng.standard_normal((H, H), np.float32) * 0.02,
        "bo": np.zeros(H, np.float32),
    }
    out = kernel(**inputs)
    print("out", out.shape, out.dtype)
